# revision 1
# baseline (speedup 1.0000x reference)
"""DenseDilatedKnnGraph Trainium2 Bass kernel — two-level top-k rewrite.

Per core (batch b = c//2, query half h = c%2): 4096 queries x 8192
candidates. Scores s = 2e - sq_n - sq_m (bitwise-matching the jax f32
chain). Instead of 11 full-row DVE passes (baseline), each 128-query
block does:

  1. tch = 2e - sq_n via PE matmul + Act bias      (bitwise)
  2. segmented max over 128 segments of 64 (1 DVE pass over 8192)
  3. top-32 segments (11 cheap passes over 128)
     -- selection ranks by tch, not tch - sq_m; points are L2-normalized
        so sq_m = 1 +- 1e-6 and misselection needs a <2e-6 segment-max
        gap: measure-zero, and the rel-err budget absorbs it.
  4. gather the 32 selected segments of scores (tch, via HBM staging)
     and of sq_m -> G [128, 2048]; G -= sq_m  (bitwise final scores)
  5. top-32 of G (11 passes over 2048) + u16 index fixup -> global ids

DVE work per block drops ~2.6x; gathers/transposes hide under DVE on
the DMA/PE/Act engines. center-idx half of the output is pure arange,
assembled host-side.
"""
import sys
sys.path.insert(0, '/opt/trn_rl_repo')
import numpy as np

_CACHE = {}

B, C, N = 4, 16, 8192
QPC = N // 2          # queries per core
NBLK = QPC // 128     # 32 query blocks per core
NCHUNK = N // 512     # 16 candidate chunks per block row
NSEG = 128            # score segments per row (of 64 elements)
SEGL = 64
NEG = -1e30


def _build():
    import concourse.bass as bass
    import concourse.mybir as mybir
    import concourse.tile as tile
    from concourse import bacc
    from concourse import library_config

    F32 = mybir.dt.float32
    I16 = mybir.dt.int16
    U16 = mybir.dt.uint16
    U32 = mybir.dt.uint32
    AF = mybir.ActivationFunctionType
    X = mybir.AxisListType.X
    OP = mybir.AluOpType

    nc = bacc.Bacc("TRN2", target_bir_lowering=False, debug=False,
                   num_devices=8, num_swdge_queues=2)

    xbT_d = nc.dram_tensor("xbT", [N, C], F32, kind="ExternalInput")
    xqT_d = nc.dram_tensor("xqT", [QPC, C], F32, kind="ExternalInput")
    idm_d = nc.dram_tensor("idm", [128, 128], F32, kind="ExternalInput")
    qc_d = nc.dram_tensor("qc", [128, 256], I16, kind="ExternalInput")
    lid_o = nc.dram_tensor("lid_out", [QPC, 16], U16, kind="ExternalOutput")
    seg_o = nc.dram_tensor("seg_out", [QPC, 32], U16, kind="ExternalOutput")

    stage_d = nc.dram_tensor("stage", [NBLK, 128, N], F32, kind="Internal")
    sqflat_d = nc.dram_tensor("sqflat", [NSEG, SEGL], F32, kind="Internal")
    istF_d = nc.dram_tensor("istF", [NBLK, 16, 256], I16, kind="Internal")

    with tile.TileContext(nc) as tc:
        with tc.tile_pool(name="per", bufs=1) as per, \
             tc.tile_pool(name="nrm", bufs=3) as nrm, \
             tc.tile_pool(name="sco", bufs=2) as sco, \
             tc.tile_pool(name="wk", bufs=2) as wk, \
             tc.tile_pool(name="car", bufs=4) as car, \
             tc.tile_pool(name="ps", bufs=2, space="PSUM") as ps, \
             tc.tile_pool(name="pst", bufs=2, space="PSUM") as pst:

            nc.gpsimd.load_library(library_config.mlp)

            ident = per.tile([128, 128], F32)
            nc.sync.dma_start(ident[:], idm_d[:])
            qc = per.tile([128, 256], I16)
            nc.sync.dma_start(qc[:], qc_d[:])

            xnT = per.tile([16, N], F32)      # normalized candidates, C x N
            wT = per.tile([16, QPC], F32)     # normalized queries, C x Q
            nsqQ = per.tile([128, NBLK], F32)  # -sq_n per query block

            def normalize_pair(src_dram, g, nm):
                # load 2 point-major tiles as [128, 2, C], L2-normalize
                # over C per lane; returns [128, 2, 17] (c 0..15 = xn,
                # c 16 = sq). Per-lane op chain matches the XLA-on-cpu
                # lowering bitwise; pairing only batches instructions.
                xt = nrm.tile([128, 2, C], F32, tag="xt", name=f"xt{nm}")
                nc.sync.dma_start(
                    xt[:], src_dram[256 * g:256 * (g + 1), :]
                    .rearrange("(t p) c -> p t c", t=2))
                xnsq = nrm.tile([128, 2, C + 1], F32, tag="xnsq", name=f"xnsq{nm}")
                xx = nrm.tile([128, 2, C], F32, tag="xx", name=f"xx{nm}")
                nc.vector.tensor_mul(xx[:], xt[:], xt[:])
                s1 = nrm.tile([128, 2], F32, tag="s1", name=f"s1{nm}")
                nc.vector.reduce_sum(s1[:], xx[:], axis=X)
                nrm_t = nrm.tile([128, 2], F32, tag="nrm", name=f"nrm{nm}")
                nc.scalar.activation(nrm_t[:], s1[:], AF.Sqrt)
                nc.vector.tensor_scalar_max(nrm_t[:], nrm_t[:], 1e-12)
                rcp = nrm.tile([128, 2], F32, tag="rcp", name=f"rcp{nm}")
                nc.vector.reciprocal(rcp[:], nrm_t[:])
                nc.vector.tensor_mul(xnsq[:, :, 0:C], xt[:],
                                     rcp[:].unsqueeze(2).to_broadcast((128, 2, C)))
                pp = nrm.tile([128, 2, C], F32, tag="pp", name=f"pp{nm}")
                nc.vector.tensor_mul(pp[:], xnsq[:, :, 0:C], xnsq[:, :, 0:C])
                nc.vector.reduce_sum(xnsq[:, :, C:C + 1].squeeze(2), pp[:], axis=X)
                return xnsq

            # Phase A: candidates -> xnT, sqflat (HBM)
            for g in range(N // 256):
                xnsq = normalize_pair(xbT_d, g, f"b{g}")
                for u in range(2):
                    t = 2 * g + u
                    trs = pst.tile([32, 128], F32, tag="tr", name=f"trs{t}")
                    nc.tensor.transpose(trs[0:C, :], xnsq[:, u, 0:C], ident[:])
                    nc.scalar.copy(xnT[:, 128 * t:128 * (t + 1)], trs[0:C, :])
                    trs2 = pst.tile([32, 128], F32, tag="tr", name=f"trs2{t}")
                    nc.tensor.transpose(trs2[0:1, :], xnsq[:, u, C:C + 1], ident[:])
                    sqr = nrm.tile([1, 128], F32, tag="sqr", name=f"sqr{t}")
                    nc.scalar.copy(sqr[:], trs2[0:1, :])
                    nc.sync.dma_start(
                        sqflat_d[2 * t:2 * t + 2, :].unsqueeze(0),
                        sqr[:].rearrange("a (r l) -> a r l", l=SEGL))

            # Phase B: queries -> wT, nsqQ
            for g in range(QPC // 256):
                xnsq = normalize_pair(xqT_d, g, f"q{g}")
                nc.vector.tensor_scalar_mul(
                    nsqQ[:, 2 * g:2 * g + 2],
                    xnsq[:, :, C:C + 1].squeeze(2), -1.0)
                for u in range(2):
                    t = 2 * g + u
                    trs = pst.tile([32, 128], F32, tag="tr", name=f"trsq{t}")
                    nc.tensor.transpose(trs[0:C, :], xnsq[:, u, 0:C], ident[:])
                    nc.scalar.copy(wT[:, 128 * t:128 * (t + 1)], trs[0:C, :])

            def emit_sel(i):
                """matmul + act -> tch; segment select; build idx tiles;
                launch gathers. Returns handles needed by emit_fin."""
                tch = sco.tile([128, N], F32, tag="tch", name=f"tch{i}")
                for j in range(NCHUNK):
                    pe = ps.tile([128, 512], F32, tag="pe", name=f"pe{i}_{j}")
                    nc.tensor.matmul(pe[:], wT[:, 128 * i:128 * (i + 1)],
                                     xnT[:, 512 * j:512 * (j + 1)],
                                     start=True, stop=True)
                    nc.scalar.activation(tch[:, 512 * j:512 * (j + 1)], pe[:],
                                         AF.Identity, bias=nsqQ[:, i:i + 1],
                                         scale=2.0)
                    nc.sync.dma_start(stage_d[i][:, 512 * j:512 * (j + 1)],
                                      tch[:, 512 * j:512 * (j + 1)])

                segM = wk.tile([128, NSEG], F32, tag="segM", name=f"segM{i}")
                nc.vector.reduce_max(segM[:],
                                     tch[:].rearrange("p (s l) -> p s l", l=SEGL),
                                     axis=X)
                mxv = wk.tile([128, 8], F32, tag="mxv", name=f"mxv{i}")
                segidx = car.tile([128, 32], U16, tag="segidx", name=f"segidx{i}")
                for r in range(4):
                    nc.vector.max(mxv[:], segM[:])
                    nc.vector.max_index(segidx[:, 8 * r:8 * r + 8], mxv[:], segM[:])
                    if r < 3:
                        nc.vector.match_replace(segM[:], mxv[:], segM[:], NEG)

                # idx tiles: T2[p, j*8+t] = segidx[16t+p, j] replicated x8,
                # T1 = T2 + 128*q. Built via PE transposes (no gpsimd).
                segf = wk.tile([128, 32], F32, tag="segf", name=f"segf{i}")
                nc.vector.tensor_copy(segf[:], segidx[:])
                pstX = pst.tile([32, 128], F32, tag="tr", name=f"pstX{i}")
                nc.tensor.transpose(pstX[:], segf[:], ident[:])
                Xs = wk.tile([32, 128], F32, tag="Xs", name=f"Xs{i}")
                nc.scalar.copy(Xs[:], pstX[:])
                pst2 = pst.tile([16, 256], F32, tag="pst2", name=f"pst2{i}")
                for t in range(8):
                    nc.tensor.transpose(pst2[:, 32 * t:32 * (t + 1)],
                                        Xs[:, 16 * t:16 * (t + 1)],
                                        ident[0:32, 0:32])
                T2a = wk.tile([16, 256], I16, tag="T2a", name=f"T2a{i}")
                nc.vector.tensor_copy(T2a[:].rearrange("p (j t) -> p j t", t=8),
                                      pst2[:].rearrange("p (t j) -> p j t", t=8))
                nc.sync.dma_start(istF_d[i], T2a[:])
                T2 = wk.tile([128, 256], I16, tag="T2", name=f"T2_{i}")
                for r in range(8):
                    nc.sync.dma_start(T2[16 * r:16 * (r + 1), :], istF_d[i])
                T1 = wk.tile([128, 256], I16, tag="T1", name=f"T1_{i}")
                nc.vector.tensor_add(T1[:], T2[:], qc[:])

                G1 = car.tile([128, 32, SEGL], F32, tag="G1", name=f"G1_{i}")
                G2 = car.tile([128, 32, SEGL], F32, tag="G2", name=f"G2_{i}")
                src1 = stage_d[i].rearrange("q (s l) -> (q s) l", l=SEGL)
                for g in range(4):
                    nc.gpsimd.dma_gather(
                        G1[:, 8 * g:8 * (g + 1), :], src1,
                        T1[:, 64 * g:64 * (g + 1)], num_idxs=1024,
                        num_idxs_reg=1024, elem_size=SEGL, queue_num=0)
                    nc.gpsimd.dma_gather(
                        G2[:, 8 * g:8 * (g + 1), :], sqflat_d[:],
                        T2[:, 64 * g:64 * (g + 1)], num_idxs=1024,
                        num_idxs_reg=1024, elem_size=SEGL, queue_num=1)
                return dict(G1=G1, G2=G2, segidx=segidx)

            def emit_fin(i, h):
                """G = G1 - sq_m (bitwise scores); top-32; u16 fixup to
                global candidate ids; write even ranks."""
                Gs = h["G1"][:].rearrange("p a b -> p (a b)")
                nc.vector.tensor_sub(Gs, Gs, h["G2"][:].rearrange("p a b -> p (a b)"))
                Gp = wk.tile([128, 2048], F32, tag="Gp", name=f"Gp{i}")
                nc.sync.dma_start(Gp[:], Gs)
                gmxa = wk.tile([128, 32], F32, tag="gmxa", name=f"gmxa{i}")
                for r in range(4):
                    gmx = gmxa[:, 8 * r:8 * r + 8]
                    nc.vector.max(gmx, Gs)
                    if r < 3:
                        nc.vector.match_replace(Gs, gmx, Gs, NEG)
                lidx = wk.tile([128, 16], U16, tag="lidx", name=f"lidx{i}")
                for k in range(2):
                    nc.vector.max_index(lidx[:, 8 * k:8 * k + 8],
                                        gmxa[:, 16 * k:16 * k + 16:2], Gp[:])
                nc.sync.dma_start(lid_o[128 * i:128 * (i + 1), :], lidx[:])
                nc.sync.dma_start(seg_o[128 * i:128 * (i + 1), :], h["segidx"][:])

            LAG = 3
            handles = {}
            for i in range(NBLK + LAG):
                if i < NBLK:
                    handles[i] = emit_sel(i)
                if i >= LAG:
                    emit_fin(i - LAG, handles.pop(i - LAG))

    nc.compile()
    return nc


def _consts():
    idm = np.eye(128, dtype=np.float32)
    P, COL = np.meshgrid(np.arange(128), np.arange(256), indexing='ij')
    qc = (128 * (16 * (COL % 8) + (P % 16))).astype(np.int16)
    return idm, qc


def _get_nc():
    if 'nc' not in _CACHE:
        _CACHE['nc'] = _build()
    return _CACHE['nc']


def kernel(x) -> np.ndarray:
    from concourse.bass_utils import run_bass_kernel_spmd

    x = np.asarray(x)
    assert x.shape == (B, C, N, 1) and x.dtype == np.float32
    xs = x[:, :, :, 0]  # (B, C, N)
    idm, qc = _consts()

    in_maps = []
    for c in range(8):
        b, h = c // 2, c % 2
        in_maps.append({
            "xbT": np.ascontiguousarray(xs[b].T),                            # (N, C)
            "xqT": np.ascontiguousarray(xs[b, :, h * QPC:(h + 1) * QPC].T),  # (QPC, C)
            "idm": idm, "qc": qc,
        })

    nc = _get_nc()
    res = run_bass_kernel_spmd(nc, in_maps, list(range(8)))

    nn = np.empty((B, N, 16), np.int32)
    for c in range(8):
        b, h = c // 2, c % 2
        lid = res.results[c]["lid_out"].astype(np.int64)     # (QPC, 16) local in G
        seg = res.results[c]["seg_out"].astype(np.int64)     # (QPC, 32) segment ids
        g = np.take_along_axis(seg, lid >> 6, axis=1) * 64 + (lid & 63)
        nn[b, h * QPC:(h + 1) * QPC] = g.astype(np.int32)
    ctr = np.broadcast_to(np.arange(N, dtype=np.int32)[None, :, None],
                          (B, N, 16)).copy()
    return np.stack([nn, ctr], axis=0)  # (2, B, N, 16) int32

